# revision 8
# baseline (speedup 1.0000x reference)
"""Multi-head causal self-attention on 8 Trainium2 NeuronCores (Bass/Tile).

Problem: x[2,2048,1024], 16 heads, d_k=64, causal softmax, out-proj + bias.

Sharding (tensor-parallel over heads + data-parallel over batch):
  core c in 0..7: batch b = c//4, heads 4*(c%4) .. 4*(c%4)+3.
  Each core: projections (f32r), per-head causal attention computed in the
  transposed score layout S^T[s,t] (no max subtraction -- scores are O(3)),
  exp on ACT, P^T column sums via a ones column appended to V, per-head
  partial output projection, then ReduceScatter(add) over each 4-core group
  which also hands every core a disjoint 512-row slice of y.

All matmuls run in float32r (TF32-like: fp32 with 11-bit RNE mantissa) at
1 cycle/row. Inputs are pre-rounded on the host (bit-identical to the
device's tensor_copy rounding) so DMA feeds valid f32r directly.
"""
import sys

sys.path.insert(0, "/opt/trn_rl_repo")

import numpy as np
import concourse.bass as bass
import concourse.mybir as mybir
from concourse.bass_utils import run_bass_kernel_spmd
from concourse.tile import TileContext

FP32 = mybir.dt.float32
F32R = mybir.dt.float32r
BF16 = mybir.dt.bfloat16

B, T, C = 2, 2048, 1024
H, DK = 16, 64
NCORES = 8
HPC = 4            # heads per core
TB = T // 128      # 16 t-blocks
CB = C // 128      # 8 channel blocks
NCHUNK = T // 512  # 4 512-col chunks
GROUPS = [[0, 1, 2, 3], [4, 5, 6, 7]]

_CACHE = {}


def _split_excess_waits(nc):
    """This walrus build encodes at most ONE sync wait per instruction.
    Hoist extras onto same-engine nops placed just before."""
    ctr = 0
    for f in nc.m.functions:
        for bb in f.blocks:
            new_insts = []
            changed = False
            for inst in bb.instructions:
                si = inst.sync_info
                if si is not None and si.on_wait and len(si.on_wait) > 1:
                    waits = list(si.on_wait)
                    for w in waits[:-1]:
                        ctr += 1
                        nop = mybir.InstNoOp(
                            name=f"I-waitsplit-{ctr}", ins=[], outs=[]
                        )
                        nop.engine = inst.engine
                        nop.sync_info = mybir.SyncInfo(on_wait=[w], on_update=[])
                        new_insts.append(nop)
                        changed = True
                    inst.sync_info = mybir.SyncInfo(
                        on_wait=[waits[-1]],
                        on_update=list(si.on_update) if si.on_update else [],
                    )
                new_insts.append(inst)
            if changed:
                bb.instructions = new_insts
    return ctr


def _rnd11(a):
    """Round fp32 array to the f32r grid (RNE to 11 mantissa bits) --
    bit-identical to the device's fp32->f32r tensor_copy."""
    a = np.ascontiguousarray(a, dtype=np.float32)
    b = a.view(np.uint32).astype(np.uint64)
    out = (((b + (((b >> 12) & 1) + 0x7FF)) >> 12) << 12).astype(np.uint32)
    return out.view(np.float32)


def build_program(n_reps=1, loop_always=False):
    """Emit the SPMD Bass program (same NEFF on all 8 cores).

    n_reps > 1 repeats the whole compute body (for benchmarking: the
    per-rep delta between two NEFF variants isolates HW time from launch
    overhead).
    """
    nc = bass.Bass("TRN2", target_bir_lowering=False, debug=False,
                   num_devices=NCORES)

    xt = nc.declare_dram_parameter("xt", [C, T], F32R, isOutput=False)
    wq = nc.declare_dram_parameter("wq", [C, HPC * DK], F32R, isOutput=False)
    wk = nc.declare_dram_parameter("wk", [C, HPC * DK], F32R, isOutput=False)
    wv = nc.declare_dram_parameter("wv", [C, HPC * DK], F32R, isOutput=False)
    wot = nc.declare_dram_parameter("wot", [HPC * DK, C], F32R, isOutput=False)
    bo4 = nc.declare_dram_parameter("bo4", [1, C], F32R, isOutput=False)
    maskp = nc.declare_dram_parameter("maskp", [128, 128], FP32, isOutput=False)
    y = nc.declare_dram_parameter("y", [T // 4, C], FP32, isOutput=True)

    yp = nc.dram_tensor("yp", [T, C], FP32)       # partial y, pre-reduce
    ys = nc.dram_tensor("ys", [T // 4, C], FP32)  # my scattered slice

    with TileContext(nc) as tc:
        with (
            tc.tile_pool(name="const", bufs=1) as pc,
            tc.tile_pool(name="qk", bufs=1) as pqk,
            tc.tile_pool(name="v5p", bufs=1) as pv5,
            tc.tile_pool(name="outp", bufs=1) as pout,
        ):
            maskt = pc.tile([128, 128], FP32)
            nc.sync.dma_start(out=maskt[:], in_=maskp[:])
            onesb = pc.tile([128, 64], FP32)
            nc.vector.memset(onesb[:], 1.0)
            onescol = pc.tile([1, 128], FP32)
            nc.vector.memset(onescol[:], 1.0)
            bot = pc.tile([1, C], F32R)
            nc.sync.dma_start(out=bot[:], in_=bo4[:])

            # long-lived activation tiles
            QT = [pqk.tile([128, T], F32R, tag=f"qt{p}", name=f"qt{p}") for p in range(2)]
            KT = [pqk.tile([128, T], F32R, tag=f"kt{p}", name=f"kt{p}") for p in range(2)]
            V5 = [pv5.tile([128, HPC * 65], F32R, tag=f"v5_{tt}", name=f"v5_{tt}")
                  for tt in range(TB)]
            OUTT = [pout.tile([64, T], F32R, tag=f"out{h}", name=f"out{h}") for h in range(HPC)]

            if n_reps == 1 and not loop_always:
                _emit_body(nc, tc, xt, wq, wk, wv, wot, bot, maskt,
                           onesb, onescol, QT, KT, V5, OUTT, yp, ys, y)
            else:
                with tc.For_i(0, n_reps, 1) as _i:
                    _emit_body(nc, tc, xt, wq, wk, wv, wot, bot, maskt,
                               onesb, onescol, QT, KT, V5, OUTT, yp, ys, y)
            _emit_ccout(nc, tc, yp, ys, y)

    _split_excess_waits(nc)
    return nc


def _emit_body(nc, tc, xt, wq, wk, wv, wot, bot, maskt, onesb, onescol,
               QT, KT, V5, OUTT, yp, ys, y):
    # ---------------- Phase A: projections ----------------
    with (
        tc.tile_pool(name="xtp", bufs=1) as px,
        tc.tile_pool(name="wp", bufs=1) as pw,
        tc.tile_pool(name="psA", bufs=2, space="PSUM") as ppa,
    ):
        XT = []
        for cb in range(CB):
            t = px.tile([128, T], F32R, tag=f"xt{cb}")
            nc.sync.dma_start(out=t[:], in_=xt[cb * 128:(cb + 1) * 128, :])
            XT.append(t)
        WQ, WK, WV = [], [], []
        for nm, dst, src in (("q", WQ, wq), ("k", WK, wk), ("v", WV, wv)):
            for cb in range(CB):
                t = pw.tile([128, HPC * DK], F32R, tag=f"w{nm}{cb}")
                nc.sync.dma_start(out=t[:], in_=src[cb * 128:(cb + 1) * 128, :])
                dst.append(t)

        # q/k projections, head-pair packed (M=128):
        # QT[p][(h%2)*64 + d, t] = q_{2p+h%2}[t, d]
        for W, DST in ((WQ, QT), (WK, KT)):
            for p in range(2):
                for tch in range(NCHUNK):
                    ps = ppa.tile([128, 512], FP32, tag="psqk")
                    for cb in range(CB):
                        nc.tensor.matmul(
                            ps[:],
                            W[cb][:, p * 128:(p + 1) * 128],
                            XT[cb][:, tch * 512:(tch + 1) * 512],
                            start=(cb == 0), stop=(cb == CB - 1))
                    nc.vector.tensor_copy(
                        DST[p][:, tch * 512:(tch + 1) * 512], ps[:])

        # v projection -> V5 tiles [128, 4*65]; per head block: v cols then ones
        for tt in range(TB):
            ps = ppa.tile([128, HPC * DK], FP32, tag="psv")
            for cb in range(CB):
                nc.tensor.matmul(
                    ps[:],
                    XT[cb][:, tt * 128:(tt + 1) * 128],
                    WV[cb][:],
                    start=(cb == 0), stop=(cb == CB - 1))
            for h in range(HPC):
                nc.vector.tensor_copy(
                    V5[tt][:, 65 * h:65 * h + 64],
                    ps[:, h * 64:(h + 1) * 64])
                nc.scalar.copy(
                    V5[tt][:, 65 * h + 64:65 * h + 65], onesb[:, 0:1])

    # ---------------- Phase B: per-head causal attention ----------------
    with (
        tc.tile_pool(name="ptp", bufs=3) as ppt,
        tc.tile_pool(name="stg", bufs=2) as pst,
        tc.tile_pool(name="psS", bufs=2, space="PSUM") as pps,
        tc.tile_pool(name="psO", bufs=1, space="PSUM") as ppo,
        tc.tile_pool(name="psB", bufs=1, space="PSUM") as ppb,
    ):
        for h in range(HPC):
            p, r0 = h // 2, (h % 2) * 64
            qh = QT[p][r0:r0 + 64, :]
            kh = KT[p][r0:r0 + 64, :]
            ps_oc = [ppo.tile([65, 512], FP32, tag=f"oc{c}", name=f"oc{c}")
                     for c in range(NCHUNK)]
            for jj in range(TB):
                cs0 = 512 * (jj // 4)          # PT col base (global t)
                gap = 128 * (jj % 4)           # zero cols [cs0, jj*128)
                PT = ppt.tile([128, T - cs0], F32R, tag="pt")
                if gap:
                    nc.vector.memset(PT[:, 0:gap].bitcast(FP32), 0.0)
                # S^T chunks for s-block jj: t in [jj*128, 2048)
                col = jj * 128
                first = True
                while col < T:
                    w = min(512 - (col % 512), T - col)
                    ps_s = pps.tile([128, 512], FP32, tag="pss")
                    nc.tensor.matmul(
                        ps_s[:, 0:w],
                        kh[:, jj * 128:(jj + 1) * 128],
                        qh[:, col:col + w],
                        start=True, stop=True)
                    if first:
                        # diagonal block: causal mask (adds -1e9 above diag)
                        nc.vector.tensor_tensor(
                            out=ps_s[:, 0:128], in0=ps_s[:, 0:128],
                            in1=maskt[:], op=mybir.AluOpType.add)
                        first = False
                    nc.scalar.activation(
                        PT[:, col - cs0:col - cs0 + w], ps_s[:, 0:w],
                        mybir.ActivationFunctionType.Exp, scale=0.125)
                    col += w
                # attn@V contributions of s-block jj to all open chunks
                for c in range(jj // 4, NCHUNK):
                    nc.tensor.matmul(
                        ps_oc[c][:],
                        V5[jj][:, 65 * h:65 * h + 65],
                        PT[:, c * 512 - cs0:(c + 1) * 512 - cs0],
                        start=(jj == 0), stop=(jj == 4 * c + 3))
                    if jj == 4 * c + 3:
                        # close chunk c: divide rows 0..63 by sums (row 64)
                        raw = pst.tile([64, 512], FP32, tag="raw")
                        nc.scalar.copy(raw[:], ps_oc[c][0:64, :])
                        rect = pst.tile([128, 512], F32R, tag="rect")
                        with nc.allow_low_precision(reason="f32r recip"):
                            nc.vector.reciprocal(
                                rect[64:65, :], ps_oc[c][64:65, :])
                        psb = ppb.tile([64, 512], FP32, tag="bc")
                        nc.tensor.matmul(
                            psb[:],
                            onesb[64:65, :].bitcast(F32R),
                            rect[64:65, :],
                            start=True, stop=True)
                        with nc.allow_low_precision(reason="f32r out"):
                            nc.vector.tensor_tensor(
                                out=OUTT[h][:, c * 512:(c + 1) * 512],
                                in0=raw[:], in1=psb[:],
                                op=mybir.AluOpType.mult)

    # ---------------- Phase C: partial out-proj + ReduceScatter ----------------
    with (
        tc.tile_pool(name="wotp", bufs=1) as pwo,
        tc.tile_pool(name="ysb", bufs=3) as pys,
        tc.tile_pool(name="psC", bufs=2, space="PSUM") as ppc,
    ):
        WOT = []
        for h in range(HPC):
            t = pwo.tile([64, C], F32R, tag=f"wot{h}")
            nc.sync.dma_start(out=t[:], in_=wot[h * 64:(h + 1) * 64, :])
            WOT.append(t)
        for tblk in range(TB):
            for dc in range(2):
                ps_y = ppc.tile([128, 512], FP32, tag="psy")
                for h in range(HPC):
                    nc.tensor.matmul(
                        ps_y[:],
                        OUTT[h][:, tblk * 128:(tblk + 1) * 128],
                        WOT[h][:, dc * 512:(dc + 1) * 512],
                        start=(h == 0), stop=False)
                nc.tensor.matmul(
                    ps_y[:],
                    onescol[0:1, :].bitcast(F32R),
                    bot[0:1, dc * 512:(dc + 1) * 512],
                    start=False, stop=True)
                ysb = pys.tile([128, 512], FP32, tag="ysb")
                nc.scalar.copy(ysb[:], ps_y[:])
                nc.sync.dma_start(
                    out=yp[tblk * 128:(tblk + 1) * 128,
                           dc * 512:(dc + 1) * 512],
                    in_=ysb[:])



def _make_in_maps(x, Wq, Wk, Wv, Wo, bo):
    mask = np.where(
        np.arange(128)[:, None] <= np.arange(128)[None, :], 0.0, -1e9
    ).astype(np.float32)
    in_maps = []
    for c in range(NCORES):
        b, hh = c // 4, HPC * (c % 4)
        ch0 = hh * DK
        in_maps.append({
            "xt": _rnd11(x[b].T),
            "wq": _rnd11(np.concatenate([Wq[hh + i] for i in range(HPC)], axis=1)),
            "wk": _rnd11(np.concatenate([Wk[hh + i] for i in range(HPC)], axis=1)),
            "wv": _rnd11(np.concatenate([Wv[hh + i] for i in range(HPC)], axis=1)),
            "wot": _rnd11(Wo[:, ch0:ch0 + HPC * DK].T),
            "bo4": _rnd11((bo / 4.0).reshape(1, C)),
            "maskp": mask,
        })
    return in_maps


def kernel(x, Wq, Wk, Wv, Wo, bo):
    x = np.asarray(x, dtype=np.float32)
    Wq = np.asarray(Wq, dtype=np.float32)
    Wk = np.asarray(Wk, dtype=np.float32)
    Wv = np.asarray(Wv, dtype=np.float32)
    Wo = np.asarray(Wo, dtype=np.float32)
    bo = np.asarray(bo, dtype=np.float32)

    if "nc" not in _CACHE:
        _CACHE["nc"] = build_program()
    nc = _CACHE["nc"]

    in_maps = _make_in_maps(x, Wq, Wk, Wv, Wo, bo)
    res = run_bass_kernel_spmd(nc, in_maps, list(range(NCORES)))

    out = np.empty((B, T, C), dtype=np.float32)
    for c in range(NCORES):
        b, r = c // 4, c % 4
        out[b, r * 512:(r + 1) * 512, :] = res.results[c]["y"]
    return out


def _emit_ccout(nc, tc, yp, ys, y):
    with tc.tile_pool(name="ccb", bufs=2) as pcc:
        nc.gpsimd.collective_compute(
            "ReduceScatter", mybir.AluOpType.add,
            ins=[yp[:]], outs=[ys[:]], replica_groups=GROUPS)
        # ys -> external output (bounce through SBUF)
        for i in range(4):
            t = pcc.tile([128, C], FP32, tag="ybounce")
            nc.sync.dma_start(out=t[:], in_=ys[i * 128:(i + 1) * 128, :])
            nc.sync.dma_start(out=y[i * 128:(i + 1) * 128, :], in_=t[:])
